# revision 1
# baseline (speedup 1.0000x reference)
"""Trainium2 Bass kernel for nn_CustomLoss_62921270887106.

Loss = BCE(class_pred, class_gt) (mean, torch log-clamp at -100)
     + mean_b( 0.5 * sum_jc[ (class_pred>=0.5) * (reg_pred-reg_gt)^2 ] / (1 + sum_j class_gt) )

Strategy: pure data parallel over the batch dim on 8 NeuronCores.
Each core reduces its 125000-sample shard to per-partition partial sums
[128, 2] (col0: sum of BCE log-terms, col1: sum of 0.5*sq/nj); the host
sums the 8x128 partials in float64 and combines.

Key per-core pipeline (sample-major layout, K=61 samples per partition
per tile, 16 main tiles of 7808 samples + one 72-sample tail tile):
  u    = (p - 1) + g                      [DVE scalar_tensor_tensor]
  t    = |u|  (== p if g==1 else 1-p)     [ACT Abs]
  L    = ln(t + 2e-38), accum -> bce col  [ACT Ln with accum_out]
  diff = rp - rg                          [DVE tensor_sub]
  d2   = diff^2                           [ACT Square]
  md   = (p >= 0.5) * d2                  [DVE scalar_tensor_tensor, is_ge+mult]
  sq   = reduce_X md  [128,61,34]->[128,61]
  njs  = reduce_X g   [128,61,17]->[128,61]
Epilogue: 1/nj via exp(-ln(nj)) on ACT, 0.5*sq*rnj via tensor_tensor_reduce.

Optimization notes (measured on HW, core0 exec time):
  - this exact structure: 164.2us. DMA engines 80% busy on the single
    sync HWDGE queue (~0.7us descriptor-gen gap per DMA instruction);
    DVE ~140us busy (stt pays a shared-SBUF-port tax vs gpsimd's sub).
  - dual-queue variants (tt stream on the scalar HWDGE queue, with
    dense-mask/bf16/software-pipelining): 232/201/202/218us - the
    ACT-issued DMAs head-of-line block on buffer-release sems and the
    extra DMA/compute overlap amplifies SBUF port contention.
  - single-queue restructure (dense mask via ACT Copy + bf16 chain +
    emission pipelining): 184.9us.
  - paired 24.9KB DMAs (8 instead of 32, bufs=2 pairs): 226.6us -
    the halved buffer runway starves the bus in 10-36us bursts.
  - e buffer in PSUM (to dodge the stt port tax): 184.2us.
  - last tile as 4 slices for a shorter drain: 167.2us (6 extra DMA
    queue gaps outweigh the ~2us drain saving).
  - tt stream on the gpsimd SWDGE queue (issue waits pre-satisfied):
    236.7us - the DVE mask stt degrades to 4.8us avg against the
    extra concurrent SBUF write stream.
  - sub as gpsimd scalar_tensor_tensor (hoping for a faster ucode
    impl than tensor_sub's 0.42-efficiency Add): fails to compile
    (CallFunctionObjArgs in the NEFF lowering; gpsimd stt unsupported).
  - inp bufs=5 on this exact structure (one more tile of DMA runway):
    187.8us - same overlap-tax law, more concurrent DMA writes slow
    the DVE ops more than the queue gains.
  - K=71 tiling (30 DMA instructions instead of 35, one K=53 remainder
    tile): 194.4us - longer per-tile chains and collision windows.
  Conclusion: the kernel is pinned by SBUF port contention around the
  DVE 2-port mask op; the single-queue DMA pacing that leaves the DMA
  engines 20% idle is what keeps DVE flowing. Every overlap-increasing
  change measured worse. This file keeps the best measured
  configuration (164.2us, ~76% of the 406GB/s DMA roofline).
"""

import sys

for _p in ("/opt/trn_rl_repo",):
    if _p not in sys.path:
        sys.path.insert(0, _p)

import numpy as np

import concourse.bass as bass
import concourse.tile as tile
from concourse import bacc, mybir
from concourse.bass_utils import run_bass_kernel_spmd

F32 = mybir.dt.float32
AF = mybir.ActivationFunctionType
ALU = mybir.AluOpType
AX = mybir.AxisListType

B = 1_000_000
J = 17
C = 3
N_CORES = 8
N_LOC = B // N_CORES            # 125000 samples per core
P = 128
K = 61                          # samples per partition per main tile
M = J * C                       # 51 floats per sample

_PROGRAM_CACHE = {}


def _build_program(n_loc=N_LOC):
    TILE_SAMPLES = P * K             # 7808
    NT_MAIN = n_loc // TILE_SAMPLES
    MAIN = NT_MAIN * TILE_SAMPLES
    TAIL = n_loc - MAIN
    NCOLS = NT_MAIN * K + 1          # sq/nj buffer columns
    N_LOC_ = n_loc
    nc = bacc.Bacc("TRN2", target_bir_lowering=False, debug=False,
                   num_devices=N_CORES)

    o_dram = nc.dram_tensor("output", [N_LOC_, J, C], F32, kind="ExternalInput").ap()
    t_dram = nc.dram_tensor("target", [N_LOC_, J, C], F32, kind="ExternalInput").ap()
    partials = nc.dram_tensor("partials", [P, 2], F32, kind="ExternalOutput").ap()

    o_flat = o_dram.rearrange("b j c -> b (j c)")
    t_flat = t_dram.rearrange("b j c -> b (j c)")
    o_main = o_flat[0:MAIN, :].rearrange("(n p k) m -> n p (k m)", p=P, k=K)
    t_main = t_flat[0:MAIN, :].rearrange("(n p k) m -> n p (k m)", p=P, k=K)
    o_tail = o_flat[MAIN:N_LOC_, :]   # [72, 51]
    t_tail = t_flat[MAIN:N_LOC_, :]

    with tile.TileContext(nc) as tc:
        with (
            tc.tile_pool(name="inp", bufs=4) as inp,
            tc.tile_pool(name="work", bufs=2) as work,
            tc.tile_pool(name="persist", bufs=1) as persist,
        ):
            sqbuf = persist.tile([P, NCOLS], F32)
            njbuf = persist.tile([P, NCOLS], F32)
            bcecols = persist.tile([P, NT_MAIN + 1], F32)
            outtile = persist.tile([P, 2], F32)
            bias_one = persist.tile([P, 1], F32)

            nc.gpsimd.memset(sqbuf[:], 0.0)
            nc.gpsimd.memset(njbuf[:], 0.0)
            nc.gpsimd.memset(bcecols[:], 0.0)
            nc.gpsimd.memset(bias_one[:], 1.0)

            def do_tile(o_src, t_src, rows, k, t_idx, sq_dst, nj_dst, bce_dst):
                # o_src/t_src: DRAM APs [rows, k*M]
                to = inp.tile([P, k * M], F32, tag="to")
                tt = inp.tile([P, k * M], F32, tag="tt")
                nc.sync.dma_start(out=to[:rows, :], in_=o_src)
                nc.sync.dma_start(out=tt[:rows, :], in_=t_src)

                o4 = to[:rows, :].rearrange("p (k j c) -> p k j c", k=k, j=J, c=C)
                t4 = tt[:rows, :].rearrange("p (k j c) -> p k j c", k=k, j=J, c=C)
                p_b = o4[:, :, :, 2:3].broadcast_to([rows, k, J, 2])

                # full-width diff on gpsimd (dense in, dense out):
                # class col gets dc = p - g, and since g in {0,1}:
                # |p + g - 1| = 1 - |p - g|  -> BCE t comes from dc for free
                dfull = work.tile([P, k * M], F32, tag="dfull")
                nc.gpsimd.tensor_sub(dfull[:rows, :], to[:rows, :], tt[:rows, :])
                d4 = dfull[:rows, :].rearrange("p (k j c) -> p k j c",
                                               k=k, j=J, c=C)
                dc = d4[:, :, :, 2].rearrange("p k j -> p (k j)")  # [rows, k*J]

                # BCE: a = |dc| * (1 - 2^-23) ; L = ln(1 - a) with accum.
                # The scale keeps a < 1 strictly so ln never sees 0.
                tabs = work.tile([P, k * J], F32, tag="tabs")
                nc.scalar.activation(tabs[:rows, :], dc, AF.Abs,
                                     scale=float(1.0 - 2.0 ** -23))
                nc.scalar.activation(tabs[:rows, :], tabs[:rows, :], AF.Ln,
                                     bias=bias_one[:rows, 0:1], scale=-1.0,
                                     accum_out=bce_dst)

                # squared diff, then pair-sum over the 2 coords FIRST
                # (1-port reduce, can't be port-blocked by gpsimd), so the
                # blockable 2-port mask op runs on half the elements with
                # a plain strided in0 (no step-0 broadcast).
                d2 = work.tile([P, k, J, 2], F32, tag="d2")
                nc.scalar.activation(d2[:rows], d4[:, :, :, 0:2], AF.Square)
                p_flat = o4[:, :, :, 2].rearrange("p k j -> p (k j)")
                e = work.tile([P, k * J], F32, tag="e")
                nc.vector.tensor_reduce(
                    e[:rows, :], d2[:rows].rearrange("p k j c -> p (k j) c"),
                    axis=AX.X, op=ALU.add)
                nc.vector.scalar_tensor_tensor(
                    out=e[:rows, :], in0=p_flat, scalar=0.5, in1=e[:rows, :],
                    op0=ALU.is_ge, op1=ALU.mult,
                )
                nc.vector.tensor_reduce(
                    sq_dst, e[:rows, :].rearrange("p (k j) -> p k j", k=k),
                    axis=AX.X, op=ALU.add)
                g3 = t4[:, :, :, 2]                                     # [rows, k, J]
                nc.vector.tensor_reduce(nj_dst, g3, axis=AX.X, op=ALU.add)

            # tail first: its small serial ops hide under the pipeline ramp
            if TAIL > 0:
                do_tile(
                    o_tail, t_tail, TAIL, 1, NT_MAIN,
                    sq_dst=sqbuf[:TAIL, NCOLS - 1:NCOLS],
                    nj_dst=njbuf[:TAIL, NCOLS - 1:NCOLS],
                    bce_dst=bcecols[:TAIL, NT_MAIN:NT_MAIN + 1],
                )
            for t in range(NT_MAIN):
                do_tile(
                    o_main[t], t_main[t], P, K, t,
                    sq_dst=sqbuf[:, t * K:(t + 1) * K],
                    nj_dst=njbuf[:, t * K:(t + 1) * K],
                    bce_dst=bcecols[:, t:t + 1],
                )

            # epilogue: wsum = sum_cols sq / (2 * (1 + nj)), all in-place in njp
            njp = persist.tile([P, NCOLS], F32)
            nc.vector.tensor_scalar_add(njp[:], njbuf[:], 1.0)
            nc.scalar.activation(njp[:], njp[:], AF.Ln, scale=2.0)   # ln(2*nj)
            nc.scalar.activation(njp[:], njp[:], AF.Exp, scale=-1.0)  # 1/(2*nj)
            nc.vector.tensor_mul(njp[:], sqbuf[:], njp[:])
            nc.vector.tensor_reduce(outtile[:, 1:2], njp[:], axis=AX.X,
                                    op=ALU.add)
            nc.vector.tensor_reduce(outtile[:, 0:1], bcecols[:], axis=AX.X,
                                    op=ALU.add)
            nc.sync.dma_start(out=partials, in_=outtile[:])

    nc.compile()
    return nc


def _get_program(n_loc=N_LOC):
    if n_loc not in _PROGRAM_CACHE:
        _PROGRAM_CACHE[n_loc] = _build_program(n_loc)
    return _PROGRAM_CACHE[n_loc]


def _run_shards(output, target, trace=False, **kw):
    nc = _get_program()
    o = np.ascontiguousarray(np.asarray(output, dtype=np.float32))
    t = np.ascontiguousarray(np.asarray(target, dtype=np.float32))
    in_maps = []
    for i in range(N_CORES):
        sl = slice(i * N_LOC, (i + 1) * N_LOC)
        in_maps.append({"output": o[sl], "target": t[sl]})
    return run_bass_kernel_spmd(nc, in_maps, list(range(N_CORES)),
                                trace=trace, **kw)


def _combine(results):
    bce_sum = 0.0
    wsq_sum = 0.0
    for r in results:
        p = np.asarray(r["partials"], dtype=np.float64)
        bce_sum += p[:, 0].sum()
        wsq_sum += p[:, 1].sum()
    loss = -bce_sum / (B * J) + wsq_sum / B
    return np.float32(loss)


def kernel(output, target):
    res = _run_shards(output, target, trace=False)
    return _combine(res.results)



# revision 3
# speedup vs baseline: 1.0576x; 1.0576x over previous
"""Trainium2 Bass kernel for nn_CustomLoss_62921270887106.

Loss = BCE(class_pred, class_gt) (mean, torch log-clamp at -100)
     + mean_b( 0.5 * sum_jc[ (class_pred>=0.5) * (reg_pred-reg_gt)^2 ] / (1 + sum_j class_gt) )

Pure data parallel over the batch dim on 8 NeuronCores. Each core
reduces its 125000-sample shard to per-partition partials [128, 2]
(col0: sum of BCE log-terms, col1: sum of 0.5*sq/nj); the host sums the
8x128 partials in float64 and combines.

Per-core pipeline (sample-major, K=61 samples/partition/tile, 16 main
tiles + 72-sample tail):
  gpsimd : dfull = to - tt (full width, dense)             ~7.1us/tile
  ACT    : Abs(dc)*(1-2^-23); Ln(1-a) accum -> bce;        ~6.0us/tile
           Square(d01) -> d2 (bf16); Copy(p) -> pdense (bf16)
  DVE    : pair-reduce d2 -> e (bf16); stt (pdense>=0.5)*e; ~7.7us/tile
           seg-reduce e -> sq; seg-reduce g -> nj
Epilogue: 1/nj via exp(-ln(2nj)), wsum via mul+reduce.

Measured on HW (exec time, 8-core SPMD, this session):
  all-f32 baseline (prev session's best):        187.2us
  d2/e intermediates in bf16:                    164.5us
  + pdense (ACT Copy p -> dense bf16 stt in0):   159.8us  <- THIS FILE
DVE is 100% busy zero-gap in steady state (7.67us/tile); the bf16
intermediates work by halving byte traffic through the contended SBUF
ports, not by making individual ops faster (d2 back to f32 made the
pair-reduce op faster but regressed the kernel to 188us).
Measured dead ends this session: mask on gpsimd tensor_scalar is_ge
(16.7us/instr ucode disaster, 335us); full sub on DVE (5.6us, 2-port
ops degrade under ambient SBUF traffic); tt-stream on scalar HWDGE
queue (197us); inp bufs=5 (187us); ACT Copy g-dense for fast nj +
strided stt (193us)."""

import sys

for _p in ("/opt/trn_rl_repo",):
    if _p not in sys.path:
        sys.path.insert(0, _p)

import numpy as np

import concourse.bass as bass
import concourse.tile as tile
from concourse import bacc, mybir
from concourse.bass_utils import run_bass_kernel_spmd

F32 = mybir.dt.float32
BF16 = mybir.dt.bfloat16
AF = mybir.ActivationFunctionType
ALU = mybir.AluOpType
AX = mybir.AxisListType

B = 1_000_000
J = 17
C = 3
N_CORES = 8
N_LOC = B // N_CORES            # 125000 samples per core
P = 128
K = 61                          # samples per partition per main tile
M = J * C                       # 51 floats per sample

_PROGRAM_CACHE = {}


def _build_program(n_loc=N_LOC):
    TILE_SAMPLES = P * K             # 7808
    NT_MAIN = n_loc // TILE_SAMPLES
    MAIN = NT_MAIN * TILE_SAMPLES
    TAIL = n_loc - MAIN
    NCOLS = NT_MAIN * K + 1          # sq/nj buffer columns
    N_LOC_ = n_loc
    nc = bacc.Bacc("TRN2", target_bir_lowering=False, debug=False,
                   num_devices=N_CORES)

    o_dram = nc.dram_tensor("output", [N_LOC_, J, C], F32, kind="ExternalInput").ap()
    t_dram = nc.dram_tensor("target", [N_LOC_, J, C], F32, kind="ExternalInput").ap()
    partials = nc.dram_tensor("partials", [P, 2], F32, kind="ExternalOutput").ap()

    o_flat = o_dram.rearrange("b j c -> b (j c)")
    t_flat = t_dram.rearrange("b j c -> b (j c)")
    o_main = o_flat[0:MAIN, :].rearrange("(n p k) m -> n p (k m)", p=P, k=K)
    t_main = t_flat[0:MAIN, :].rearrange("(n p k) m -> n p (k m)", p=P, k=K)
    o_tail = o_flat[MAIN:N_LOC_, :]   # [72, 51]
    t_tail = t_flat[MAIN:N_LOC_, :]

    with tile.TileContext(nc) as tc:
        with (
            tc.tile_pool(name="inp", bufs=4) as inp,
            tc.tile_pool(name="work", bufs=2) as work,
            tc.tile_pool(name="persist", bufs=1) as persist,
        ):
            sqbuf = persist.tile([P, NCOLS], F32)
            njbuf = persist.tile([P, NCOLS], F32)
            bcecols = persist.tile([P, NT_MAIN + 1], F32)
            outtile = persist.tile([P, 2], F32)
            bias_one = persist.tile([P, 1], F32)

            nc.gpsimd.memset(sqbuf[:], 0.0)
            nc.gpsimd.memset(njbuf[:], 0.0)
            nc.gpsimd.memset(bcecols[:], 0.0)
            nc.gpsimd.memset(bias_one[:], 1.0)

            def do_tile(o_src, t_src, rows, k, t_idx, sq_dst, nj_dst, bce_dst):
                # o_src/t_src: DRAM APs [rows, k*M]
                to = inp.tile([P, k * M], F32, tag="to")
                tt = inp.tile([P, k * M], F32, tag="tt")
                nc.sync.dma_start(out=to[:rows, :], in_=o_src)
                nc.sync.dma_start(out=tt[:rows, :], in_=t_src)

                o4 = to[:rows, :].rearrange("p (k j c) -> p k j c", k=k, j=J, c=C)
                t4 = tt[:rows, :].rearrange("p (k j c) -> p k j c", k=k, j=J, c=C)
                p_b = o4[:, :, :, 2:3].broadcast_to([rows, k, J, 2])

                # full-width diff on gpsimd (dense in, dense out):
                # class col gets dc = p - g, and since g in {0,1}:
                # |p + g - 1| = 1 - |p - g|  -> BCE t comes from dc for free
                dfull = work.tile([P, k * M], F32, tag="dfull")
                nc.gpsimd.tensor_sub(dfull[:rows, :], to[:rows, :], tt[:rows, :])
                d4 = dfull[:rows, :].rearrange("p (k j c) -> p k j c",
                                               k=k, j=J, c=C)
                dc = d4[:, :, :, 2].rearrange("p k j -> p (k j)")  # [rows, k*J]

                # BCE: a = |dc| * (1 - 2^-23) ; L = ln(1 - a) with accum.
                # The scale keeps a < 1 strictly so ln never sees 0.
                tabs = work.tile([P, k * J], F32, tag="tabs")
                nc.scalar.activation(tabs[:rows, :], dc, AF.Abs,
                                     scale=float(1.0 - 2.0 ** -23))
                nc.scalar.activation(tabs[:rows, :], tabs[:rows, :], AF.Ln,
                                     bias=bias_one[:rows, 0:1], scale=-1.0,
                                     accum_out=bce_dst)

                # squared diff, then pair-sum over the 2 coords FIRST
                # (1-port reduce, can't be port-blocked by gpsimd), so the
                # blockable 2-port mask op runs on half the elements with
                # a plain strided in0 (no step-0 broadcast).
                d2 = work.tile([P, k, J, 2], BF16, tag="d2")
                nc.scalar.activation(d2[:rows], d4[:, :, :, 0:2], AF.Square)
                p_flat = o4[:, :, :, 2].rearrange("p k j -> p (k j)")
                pdense = work.tile([P, k * J], BF16, tag="pdense")
                nc.scalar.activation(pdense[:rows, :], p_flat, AF.Copy)
                e = work.tile([P, k * J], BF16, tag="e")
                with nc.allow_low_precision(reason="pair-sum of 2 bf16"):
                    nc.vector.tensor_reduce(
                        e[:rows, :],
                        d2[:rows].rearrange("p k j c -> p (k j) c"),
                        axis=AX.X, op=ALU.add)
                nc.vector.scalar_tensor_tensor(
                    out=e[:rows, :], in0=pdense[:rows, :], scalar=0.5,
                    in1=e[:rows, :], op0=ALU.is_ge, op1=ALU.mult,
                )
                nc.vector.tensor_reduce(
                    sq_dst, e[:rows, :].rearrange("p (k j) -> p k j", k=k),
                    axis=AX.X, op=ALU.add)
                g3 = t4[:, :, :, 2]                                     # [rows, k, J]
                nc.vector.tensor_reduce(nj_dst, g3, axis=AX.X, op=ALU.add)

            # tail first: its small serial ops hide under the pipeline ramp
            if TAIL > 0:
                do_tile(
                    o_tail, t_tail, TAIL, 1, NT_MAIN,
                    sq_dst=sqbuf[:TAIL, NCOLS - 1:NCOLS],
                    nj_dst=njbuf[:TAIL, NCOLS - 1:NCOLS],
                    bce_dst=bcecols[:TAIL, NT_MAIN:NT_MAIN + 1],
                )
            for t in range(NT_MAIN):
                do_tile(
                    o_main[t], t_main[t], P, K, t,
                    sq_dst=sqbuf[:, t * K:(t + 1) * K],
                    nj_dst=njbuf[:, t * K:(t + 1) * K],
                    bce_dst=bcecols[:, t:t + 1],
                )

            # epilogue: wsum = sum_cols sq / (2 * (1 + nj)), all in-place in njp
            njp = persist.tile([P, NCOLS], F32)
            nc.vector.tensor_scalar_add(njp[:], njbuf[:], 1.0)
            nc.scalar.activation(njp[:], njp[:], AF.Ln, scale=2.0)   # ln(2*nj)
            nc.scalar.activation(njp[:], njp[:], AF.Exp, scale=-1.0)  # 1/(2*nj)
            nc.vector.tensor_mul(njp[:], sqbuf[:], njp[:])
            nc.vector.tensor_reduce(outtile[:, 1:2], njp[:], axis=AX.X,
                                    op=ALU.add)
            nc.vector.tensor_reduce(outtile[:, 0:1], bcecols[:], axis=AX.X,
                                    op=ALU.add)
            nc.sync.dma_start(out=partials, in_=outtile[:])

    nc.compile()
    return nc


def _get_program(n_loc=N_LOC):
    if n_loc not in _PROGRAM_CACHE:
        _PROGRAM_CACHE[n_loc] = _build_program(n_loc)
    return _PROGRAM_CACHE[n_loc]


def _run_shards(output, target, trace=False, **kw):
    nc = _get_program()
    o = np.ascontiguousarray(np.asarray(output, dtype=np.float32))
    t = np.ascontiguousarray(np.asarray(target, dtype=np.float32))
    in_maps = []
    for i in range(N_CORES):
        sl = slice(i * N_LOC, (i + 1) * N_LOC)
        in_maps.append({"output": o[sl], "target": t[sl]})
    return run_bass_kernel_spmd(nc, in_maps, list(range(N_CORES)),
                                trace=trace, **kw)


def _combine(results):
    bce_sum = 0.0
    wsq_sum = 0.0
    for r in results:
        p = np.asarray(r["partials"], dtype=np.float64)
        bce_sum += p[:, 0].sum()
        wsq_sum += p[:, 1].sum()
    loss = -bce_sum / (B * J) + wsq_sum / B
    return np.float32(loss)


def kernel(output, target):
    res = _run_shards(output, target, trace=False)
    return _combine(res.results)



# revision 4
# speedup vs baseline: 1.2307x; 1.1636x over previous
"""Trainium2 Bass kernel for nn_CustomLoss_62921270887106.

Loss = BCE(class_pred, class_gt) (mean, torch log-clamp at -100)
     + mean_b( 0.5 * sum_jc[ (class_pred>=0.5) * (reg_pred-reg_gt)^2 ] / (1 + sum_j class_gt) )

Pure data parallel over batch on 8 NeuronCores; each core reduces its
125000-sample shard to [128,2] partials (col0 BCE log-sum, col1 weighted
MSE sum); host combines in f64.

Per-core pipeline (sample-major, K=61 samples/partition/tile, 16 main
tiles + 72-sample tail, tail emitted first):
  gpsimd: dfull = to - tt (full width, dense f32)        ~7.1us/tile
  ACT   : Abs(dc)*(1-2^-23); Ln(1-a) accum -> bce;       ~6.0us/tile
          Square -> d2 (bf16); Copy p -> pdense (bf16)
  DVE   : pair-reduce d2 -> e (bf16); (pdense>=0.5)*e    ~7.7us/tile
          via scalar_tensor_tensor; segmented add-reduces
          for per-sample sq and nj.                      <- critical
Epilogue: 1/(2(1+nj)) via exp(-ln), mul+reduce, partials DMA.

Measured (HW exec core0, 8-core SPMD, in-process reps): 159.8-161.0us
best-state; the SAME binary also runs at ~186-188us in a second device
state, traced to the 2-port stt alone going 2.5 -> 4.0us/tile (SBUF
port phase); all 1-port ops are phase-stable. All-f32 baseline: 187-194.
bf16 d2/e wins by halving SBUF port byte-traffic (not op durations:
the bf16 pair-reduce op is slower than f32, yet kernel faster; d2 back
to f32 regressed 160->188).
Measured dead ends: mask via gpsimd tensor_scalar is_ge (0.05-eff ucode,
335us); full sub on DVE (2-port fragility); dual DMA queue (197); inp
bufs=5 (187); dfull bf16 (rel err 1.5e-2 cliff); variable-K ramp tiles
(168-184); ttr epilogue (runtime crash); 1-port mask via interleaved
(e,mask)+reduce(op=mult) (216); stt out-of-place (no gain); ACT Copy g
dense for fast nj (no gain)."""

import sys

for _p in ("/opt/trn_rl_repo",):
    if _p not in sys.path:
        sys.path.insert(0, _p)

import numpy as np

import concourse.bass as bass
import concourse.tile as tile
from concourse import bacc, mybir
from concourse.bass_utils import run_bass_kernel_spmd

F32 = mybir.dt.float32
BF16 = mybir.dt.bfloat16
AF = mybir.ActivationFunctionType
ALU = mybir.AluOpType
AX = mybir.AxisListType

B = 1_000_000
J = 17
C = 3
N_CORES = 8
N_LOC = B // N_CORES            # 125000 samples per core
P = 128
K = 61                          # samples per partition per main tile
M = J * C                       # 51 floats per sample

_PROGRAM_CACHE = {}


def _build_program(n_loc=N_LOC):
    TILE_SAMPLES = P * K             # 7808
    NT_MAIN = n_loc // TILE_SAMPLES
    MAIN = NT_MAIN * TILE_SAMPLES
    TAIL = n_loc - MAIN
    NCOLS = NT_MAIN * K + 1          # sq/nj buffer columns
    N_LOC_ = n_loc
    nc = bacc.Bacc("TRN2", target_bir_lowering=False, debug=False,
                   num_devices=N_CORES)

    o_dram = nc.dram_tensor("output", [N_LOC_, J, C], F32, kind="ExternalInput").ap()
    t_dram = nc.dram_tensor("target", [N_LOC_, J, C], F32, kind="ExternalInput").ap()
    partials = nc.dram_tensor("partials", [P, 2], F32, kind="ExternalOutput").ap()

    o_flat = o_dram.rearrange("b j c -> b (j c)")
    t_flat = t_dram.rearrange("b j c -> b (j c)")
    o_main = o_flat[0:MAIN, :].rearrange("(n p k) m -> n p (k m)", p=P, k=K)
    t_main = t_flat[0:MAIN, :].rearrange("(n p k) m -> n p (k m)", p=P, k=K)
    o_tail = o_flat[MAIN:N_LOC_, :]   # [72, 51]
    t_tail = t_flat[MAIN:N_LOC_, :]

    with tile.TileContext(nc) as tc:
        with (
            tc.tile_pool(name="inp", bufs=4) as inp,
            tc.tile_pool(name="work", bufs=2) as work,
            tc.tile_pool(name="persist", bufs=1) as persist,
        ):
            sqbuf = persist.tile([P, NCOLS], F32)
            njbuf = persist.tile([P, NCOLS], F32)
            bcecols = persist.tile([P, NT_MAIN + 1], F32)
            outtile = persist.tile([P, 2], F32)
            bias_one = persist.tile([P, 1], F32)

            nc.gpsimd.memset(sqbuf[:], 0.0)
            nc.gpsimd.memset(njbuf[:], 0.0)
            nc.gpsimd.memset(bcecols[:], 0.0)
            nc.gpsimd.memset(bias_one[:], 1.0)

            def do_tile(o_src, t_src, rows, k, t_idx, sq_dst, nj_dst, bce_dst):
                # o_src/t_src: DRAM APs [rows, k*M]
                to = inp.tile([P, k * M], F32, tag="to")
                tt = inp.tile([P, k * M], F32, tag="tt")
                nc.sync.dma_start(out=to[:rows, :], in_=o_src)
                nc.sync.dma_start(out=tt[:rows, :], in_=t_src)

                o4 = to[:rows, :].rearrange("p (k j c) -> p k j c", k=k, j=J, c=C)
                t4 = tt[:rows, :].rearrange("p (k j c) -> p k j c", k=k, j=J, c=C)
                p_b = o4[:, :, :, 2:3].broadcast_to([rows, k, J, 2])

                # full-width diff on gpsimd (dense in, dense out):
                # class col gets dc = p - g, and since g in {0,1}:
                # |p + g - 1| = 1 - |p - g|  -> BCE t comes from dc for free
                dfull = work.tile([P, k * M], F32, tag="dfull")
                nc.gpsimd.tensor_sub(dfull[:rows, :], to[:rows, :], tt[:rows, :])
                d4 = dfull[:rows, :].rearrange("p (k j c) -> p k j c",
                                               k=k, j=J, c=C)
                dc = d4[:, :, :, 2].rearrange("p k j -> p (k j)")  # [rows, k*J]

                # BCE: a = |dc| * (1 - 2^-23) ; L = ln(1 - a) with accum.
                # The scale keeps a < 1 strictly so ln never sees 0.
                tabs = work.tile([P, k * J], F32, tag="tabs")
                nc.scalar.activation(tabs[:rows, :], dc, AF.Abs,
                                     scale=float(1.0 - 2.0 ** -23))
                nc.scalar.activation(tabs[:rows, :], tabs[:rows, :], AF.Ln,
                                     bias=bias_one[:rows, 0:1], scale=-1.0,
                                     accum_out=bce_dst)

                # squared diff, then pair-sum over the 2 coords FIRST
                # (1-port reduce, can't be port-blocked by gpsimd), so the
                # blockable 2-port mask op runs on half the elements with
                # a plain strided in0 (no step-0 broadcast).
                d2 = work.tile([P, k, J, 2], BF16, tag="d2")
                nc.scalar.activation(d2[:rows], d4[:, :, :, 0:2], AF.Square)
                p_flat = o4[:, :, :, 2].rearrange("p k j -> p (k j)")
                pdense = work.tile([P, k * J], BF16, tag="pdense")
                nc.scalar.activation(pdense[:rows, :], p_flat, AF.Copy)
                e = work.tile([P, k * J], BF16, tag="e")
                with nc.allow_low_precision(reason="pair-sum of 2 bf16"):
                    nc.vector.tensor_reduce(
                        e[:rows, :],
                        d2[:rows].rearrange("p k j c -> p (k j) c"),
                        axis=AX.X, op=ALU.add)
                nc.vector.scalar_tensor_tensor(
                    out=e[:rows, :], in0=pdense[:rows, :], scalar=0.5,
                    in1=e[:rows, :], op0=ALU.is_ge, op1=ALU.mult,
                )
                nc.vector.tensor_reduce(
                    sq_dst, e[:rows, :].rearrange("p (k j) -> p k j", k=k),
                    axis=AX.X, op=ALU.add)
                g3 = t4[:, :, :, 2]                                     # [rows, k, J]
                nc.vector.tensor_reduce(nj_dst, g3, axis=AX.X, op=ALU.add)

            # tail first: its small serial ops hide under the pipeline ramp
            if TAIL > 0:
                do_tile(
                    o_tail, t_tail, TAIL, 1, NT_MAIN,
                    sq_dst=sqbuf[:TAIL, NCOLS - 1:NCOLS],
                    nj_dst=njbuf[:TAIL, NCOLS - 1:NCOLS],
                    bce_dst=bcecols[:TAIL, NT_MAIN:NT_MAIN + 1],
                )
            for t in range(NT_MAIN):
                do_tile(
                    o_main[t], t_main[t], P, K, t,
                    sq_dst=sqbuf[:, t * K:(t + 1) * K],
                    nj_dst=njbuf[:, t * K:(t + 1) * K],
                    bce_dst=bcecols[:, t:t + 1],
                )

            # epilogue: wsum = sum_cols sq / (2 * (1 + nj)), all in-place in njp
            njp = persist.tile([P, NCOLS], F32)
            nc.vector.tensor_scalar_add(njp[:], njbuf[:], 1.0)
            nc.scalar.activation(njp[:], njp[:], AF.Ln, scale=2.0)   # ln(2*nj)
            nc.scalar.activation(njp[:], njp[:], AF.Exp, scale=-1.0)  # 1/(2*nj)
            nc.vector.tensor_mul(njp[:], sqbuf[:], njp[:])
            nc.vector.tensor_reduce(outtile[:, 1:2], njp[:], axis=AX.X,
                                    op=ALU.add)
            nc.vector.tensor_reduce(outtile[:, 0:1], bcecols[:], axis=AX.X,
                                    op=ALU.add)
            nc.sync.dma_start(out=partials, in_=outtile[:])

    nc.compile()
    return nc


def _get_program(n_loc=N_LOC):
    if n_loc not in _PROGRAM_CACHE:
        _PROGRAM_CACHE[n_loc] = _build_program(n_loc)
    return _PROGRAM_CACHE[n_loc]


def _run_shards(output, target, trace=False, **kw):
    nc = _get_program()
    o = np.ascontiguousarray(np.asarray(output, dtype=np.float32))
    t = np.ascontiguousarray(np.asarray(target, dtype=np.float32))
    in_maps = []
    for i in range(N_CORES):
        sl = slice(i * N_LOC, (i + 1) * N_LOC)
        in_maps.append({"output": o[sl], "target": t[sl]})
    return run_bass_kernel_spmd(nc, in_maps, list(range(N_CORES)),
                                trace=trace, **kw)


def _combine(results):
    bce_sum = 0.0
    wsq_sum = 0.0
    for r in results:
        p = np.asarray(r["partials"], dtype=np.float64)
        bce_sum += p[:, 0].sum()
        wsq_sum += p[:, 1].sum()
    loss = -bce_sum / (B * J) + wsq_sum / B
    return np.float32(loss)


def kernel(output, target):
    res = _run_shards(output, target, trace=False)
    return _combine(res.results)

